# revision 1
# baseline (speedup 1.0000x reference)
"""GPT2 attention (B=2, S=2048, E=1024, H=16, interleaved QKV) on 8 trn2 NeuronCores.

Sharding: core c = 4*b + g handles batch b = c//4 and head group g = c%4
(heads 4g..4g+3): Megatron column-split of W_attn / row-split of W_proj,
data-parallel over batch. Host sums the 4 partial projection outputs per batch.

W_attn columns are host-permuted per core to [q0..q3 | k0..k3 | v0..v3]
(64-wide blocks) so each head's q/k/v share a partition offset, which the
matmul base_partition rule requires.

Per-core dataflow (feature-on-partition layout, fp32r matmuls):
  X [2048,1024] --PE transpose--> X^T
  qkv^T = W_slice^T @ X^T   (features on partitions; q pre-scaled by 1/8)
  per head: S^T[sk,sq] = K^T-stationary matmul (causal lower-triangle blocks only)
            P = exp(S^T + mask)   (masked entries underflow to exact 0)
            PV with a ones-column appended to V -> row 64 = softmax denominators
            normalize via DVE recip + PE ones-broadcast + DVE multiply
  out^T = W_proj_slice^T-stationary matmul over the 256 local channels
"""
import numpy as np

import concourse.bass as bass
import concourse.bacc as bacc
import concourse.tile as tile
from concourse import mybir
from concourse.bass_utils import run_bass_kernel_spmd

F32 = mybir.dt.float32
F32R = mybir.dt.float32r
F16 = mybir.dt.float16

B, S, E, H = 2, 2048, 1024, 16
HD = E // H            # 64
HPC = 4                # heads per core
CW = HPC * 3 * HD      # 768: W_attn cols per core
CP = HPC * HD          # 256: W_proj rows per core
NK = E // 128          # 8 contraction chunks over E
NSQ = S // 512         # 4 sq chunks of 512
NSK = S // 128         # 16 sk chunks of 128

_cache = {}
_last_in_maps = None


def _build():
    from contextlib import ExitStack

    nc = bacc.Bacc("TRN2", target_bir_lowering=False, debug=False, num_devices=8)

    x_d = nc.dram_tensor("x", [16, 128, E], F16, kind="ExternalInput").ap()
    wa_d = nc.dram_tensor("wa", [NK, 128, CW], F16, kind="ExternalInput").ap()
    ba_d = nc.dram_tensor("ba", [6, 128, 1], F32, kind="ExternalInput").ap()
    wp_d = nc.dram_tensor("wp", [2, 128, E], F16, kind="ExternalInput").ap()
    out_d = nc.dram_tensor("out_t", [8, 128, S], F32, kind="ExternalOutput").ap()

    ident16_d = nc.inline_tensor(np.eye(128, dtype=np.float16), name="ident16")
    ones16_d = nc.inline_tensor(np.ones((128, 16, 1), np.float16), name="ones16")
    onesrow_d = nc.inline_tensor(np.ones((128, 64), np.float16), name="onesrow")

    Exp = mybir.ActivationFunctionType.Exp
    Ident = mybir.ActivationFunctionType.Identity

    with tile.TileContext(nc) as tc, ExitStack() as top:
        consts = top.enter_context(tc.tile_pool(name="consts", bufs=1))
        qk_pool = top.enter_context(tc.tile_pool(name="qkvT", bufs=1))
        at_pool = top.enter_context(tc.tile_pool(name="attnT", bufs=1))
        wp_pool = top.enter_context(tc.tile_pool(name="wp", bufs=1))

        # identity first (PE transposes need it), then stream x on the sync
        # queue; weights/consts go on the gpsimd queue so they don't delay x.
        ident16_t = consts.tile([128, 128], F16)
        nc.sync.dma_start(out=ident16_t[:], in_=ident16_d.ap())

        qkvT = [
            qk_pool.tile([128, S], F16, tag=f"qkvT{cc}", name=f"qkvT{cc}")
            for cc in range(6)
        ]
        attnT = [
            at_pool.tile([128, S], F16, tag=f"attnT{c}", name=f"attnT{c}")
            for c in range(2)
        ]

        # ---- phase 1+2: X^T then qkv^T = W^T X^T --------------------------
        with (
            tc.tile_pool(name="xio", bufs=4) as xio,
            tc.tile_pool(name="xTp", bufs=1) as xTp,
            tc.tile_pool(name="wa", bufs=1) as wa_pool,
            tc.tile_pool(name="ps_tr", bufs=4, space="PSUM") as ps_tr,
            tc.tile_pool(name="ps_mm", bufs=4, space="PSUM") as ps_mm,
        ):
            xT = [
                xTp.tile([128, S], F16, tag=f"xT{k}", name=f"xT{k}")
                for k in range(NK)
            ]
            xts = []
            for i in range(16):
                xt = xio.tile([128, E], F16, tag="xt")
                eng = nc.sync if i % 2 == 0 else nc.scalar
                eng.dma_start(out=xt[:], in_=x_d[i])
                xts.append(xt)

            wa_t = wa_pool.tile([128, NK, CW], F16)
            for k in range(NK):
                nc.gpsimd.dma_start(out=wa_t[:, k, :], in_=wa_d[k])
            wp_t = wp_pool.tile([128, 2, E], F16)
            for cc in range(2):
                nc.gpsimd.dma_start(out=wp_t[:, cc, :], in_=wp_d[cc])
            ba_t = consts.tile([128, 6], F32)
            for cc in range(6):
                nc.gpsimd.dma_start(out=ba_t[:, cc : cc + 1], in_=ba_d[cc])
            onesrow_t = consts.tile([128, 64], F16)
            nc.gpsimd.dma_start(out=onesrow_t[:], in_=onesrow_d.ap())
            # additive causal masks for the 4 diagonal-block offsets r:
            # keep (0) where f >= p + 128 r else -1e4  (p=sk in block, f=sq)
            mask_t = consts.tile([128, 4, 512], F32)
            nc.gpsimd.memset(mask_t[:], 0.0)
            for r in range(4):
                nc.gpsimd.affine_select(
                    out=mask_t[:, r, :],
                    in_=mask_t[:, r, :],
                    compare_op=mybir.AluOpType.is_ge,
                    fill=-10000.0,
                    base=-128 * r,
                    pattern=[[1, 512]],
                    channel_multiplier=-1,
                )

            for i in range(16):
                for k in range(NK):
                    ps = ps_tr.tile([128, 128], F16, tag="tr")
                    nc.tensor.transpose(
                        ps[:], xts[i][:, k * 128 : (k + 1) * 128], ident16_t[:]
                    )
                    nc.vector.tensor_copy(
                        xT[k][:, i * 128 : (i + 1) * 128], ps[:]
                    )

            for cc in range(6):
                pss = [
                    ps_mm.tile([128, 512], F32, tag="mm", name="mm_ps")
                    for _ in range(4)
                ]
                for k in range(NK):
                    lhs = wa_t[:, k, cc * 128 : (cc + 1) * 128]
                    for rc in range(4):
                        nc.tensor.matmul(
                            pss[rc][:],
                            lhs,
                            xT[k][:, rc * 512 : (rc + 1) * 512],
                            start=(k == 0),
                            stop=(k == NK - 1),
                        )
                for rc in range(4):
                    nc.scalar.activation(
                        qkvT[cc][:, rc * 512 : (rc + 1) * 512],
                        pss[rc][:],
                        Ident,
                        bias=ba_t[:, cc : cc + 1],
                        scale=0.125 if cc < 2 else 1.0,
                    )

        # ---- phase 3+4: per-head attention --------------------------------
        with (
            tc.tile_pool(name="vb", bufs=1) as vb_pool,
            tc.tile_pool(name="pp", bufs=18) as p_pool,
            tc.tile_pool(name="um", bufs=3) as u_pool,
            tc.tile_pool(name="sm", bufs=3) as small,
            tc.tile_pool(name="ps_tr2", bufs=2, space="PSUM") as ps_tr2,
            tc.tile_pool(name="ps_s", bufs=3, space="PSUM") as ps_s,
            tc.tile_pool(name="ps_bc", bufs=1, space="PSUM") as ps_bc,
            tc.tile_pool(name="ps_pv", bufs=2, space="PSUM") as ps_pv,
        ):
            # all V' tiles upfront so the attention groups never break the
            # PE stream for transposes
            vbs = []
            for h in range(HPC):
                o = (h % 2) * 64
                vT = qkvT[4 + h // 2][o : o + 64, :]
                vb = vb_pool.tile(
                    [128, 16, 65], F16, tag=f"vb{h}", name=f"vb{h}"
                )
                nc.gpsimd.dma_start(
                    out=vb[:, :, 64:65], in_=ones16_d.ap()
                )
                for i in range(NSK):
                    ps = ps_tr2.tile([128, 64], F16, tag="tr2")
                    nc.tensor.transpose(
                        ps[:],
                        vT[:, i * 128 : (i + 1) * 128],
                        ident16_t[o : o + 64, o : o + 64],
                    )
                    nc.vector.tensor_copy(vb[:, i, 0:64], ps[:])
                vbs.append(vb)

            def norm_tail(st):
                pv, rcp_r, h, J = st
                sq = bass.ts(J, 512)
                bc = ps_bc.tile([64, 512], F32, tag="bc", name="bc")
                nc.tensor.matmul(
                    bc[:],
                    onesrow_t[64:65, :],
                    rcp_r[64:65, :],
                    start=True,
                    stop=True,
                )
                u = u_pool.tile([64, 512], F32, tag="u", name="u")
                nc.scalar.copy(u[:], pv[0:64, :])
                if h % 2 == 0:
                    nc.vector.tensor_mul(attnT[h // 2][0:64, sq], u[:], bc[:])
                else:
                    tmp = u_pool.tile(
                        [64, 512], F16, tag="tmpshift", name="tmpshift"
                    )
                    nc.vector.tensor_mul(tmp[:], u[:], bc[:])
                    nc.sync.dma_start(out=attnT[h // 2][64:128, sq], in_=tmp[:])

            pending = None
            for h in range(HPC):
                o = (h % 2) * 64
                qT = qkvT[h // 2][o : o + 64, :]
                kT = qkvT[2 + h // 2][o : o + 64, :]
                vb = vbs[h]

                for J in range(NSQ):
                    nblk = 4 * J + 4
                    sq = bass.ts(J, 512)
                    pblks = []
                    for i in range(nblk):
                        sps = ps_s.tile([128, 512], F32, tag="s", name="sps")
                        nc.tensor.matmul(
                            sps[:],
                            kT[:, i * 128 : (i + 1) * 128],
                            qT[:, sq],
                            start=True,
                            stop=True,
                        )
                        r = i - 4 * J
                        if r >= 0:
                            nc.vector.tensor_add(
                                sps[:], sps[:], mask_t[:, r, :]
                            )
                        p = p_pool.tile([128, 512], F16, tag="p", name="p")
                        nc.scalar.activation(p[:], sps[:], Exp)
                        pblks.append(p)
                    # previous group's normalize tail goes here: its inputs
                    # are long since ready, so the PE takes it without a stall
                    if pending is not None:
                        norm_tail(pending)
                        pending = None
                    pv = ps_pv.tile([65, 512], F32, tag="pv", name="pv")
                    for i, p in enumerate(pblks):
                        nc.tensor.matmul(
                            pv[:],
                            vb[:, i, :],
                            p[:],
                            start=(i == 0),
                            stop=(i == nblk - 1),
                        )
                    # normalize head: DVE/ACT work that runs under the next
                    # group's matmuls. row 64 of pv = softmax denominators.
                    rcp = small.tile([128, 512], F32, tag="rcp", name="rcp")
                    nc.vector.reciprocal(rcp[64:65, :], pv[64:65, :])
                    rcp_r = small.tile([128, 512], F16, tag="rcpr", name="rcpr")
                    nc.scalar.copy(rcp_r[64:65, :], rcp[64:65, :])
                    pending = (pv, rcp_r, h, J)
            norm_tail(pending)

        # ---- phase 5: projection ------------------------------------------
        with (
            tc.tile_pool(name="ob", bufs=6) as ob_pool,
            tc.tile_pool(name="ps_mm2", bufs=4, space="PSUM") as ps_mm2,
        ):
            for eo in range(8):
                pss = [
                    ps_mm2.tile([128, 512], F32, tag="mm", name="mm_ps")
                    for _ in range(4)
                ]
                for cc in range(2):
                    lhs = wp_t[:, cc, eo * 128 : (eo + 1) * 128]
                    for J in range(4):
                        nc.tensor.matmul(
                            pss[J][:],
                            lhs,
                            attnT[cc][:, J * 512 : (J + 1) * 512],
                            start=(cc == 0),
                            stop=(cc == 1),
                        )
                for J in range(4):
                    ob = ob_pool.tile([128, 512], F32, tag="ob", name="ob")
                    nc.vector.tensor_copy(ob[:], pss[J][:])
                    eng = nc.sync if (eo * 4 + J) % 2 == 0 else nc.scalar
                    eng.dma_start(
                        out=out_d[eo][:, J * 512 : (J + 1) * 512], in_=ob[:]
                    )

    nc.compile()
    return nc


def _col_perm(g):
    """Per-core W_attn column permutation: [q0..q3 | k0..k3 | v0..v3]."""
    cols = []
    for t in range(3):          # q, k, v
        for h in range(HPC):
            base = (4 * g + h) * 3 * HD + t * HD
            cols.append(np.arange(base, base + HD))
    return np.concatenate(cols)


def kernel(hidden_states, W_attn, b_attn, W_proj, b_proj):
    hidden_states = np.asarray(hidden_states, np.float32)
    W_attn = np.asarray(W_attn, np.float32)
    b_attn = np.asarray(b_attn, np.float32)
    W_proj = np.asarray(W_proj, np.float32)
    b_proj = np.asarray(b_proj, np.float32)

    if "nc" not in _cache:
        _cache["nc"] = _build()
    nc = _cache["nc"]

    # q columns (first 256 of the permuted layout) have scale 1/8 folded into
    # the PSUM->SBUF copy; bias is added after the scale, so pre-scale it.
    bias_scale = np.ones(CW, np.float32)
    bias_scale[: 4 * HD] = 0.125

    in_maps = []
    for c in range(8):
        b, g = divmod(c, 4)
        perm = _col_perm(g)
        wa = np.ascontiguousarray(W_attn[:, perm])
        ba = (b_attn[perm] * bias_scale).astype(np.float32)
        wp = np.ascontiguousarray(W_proj[g * CP : (g + 1) * CP, :])
        in_maps.append(
            {
                "x": np.ascontiguousarray(hidden_states[b]).astype(np.float16).reshape(16, 128, E),
                "wa": wa.astype(np.float16).reshape(NK, 128, CW),
                "ba": ba.reshape(6, 128, 1),
                "wp": wp.astype(np.float16).reshape(2, 128, E),
            }
        )

    global _last_in_maps
    _last_in_maps = in_maps
    res = run_bass_kernel_spmd(nc, in_maps, list(range(8)))

    out = np.zeros((B, S, E), np.float32)
    for c in range(8):
        b = c // 4
        out[b] += res.results[c]["out_t"].reshape(E, S).T
    out += b_proj
    return out



# revision 14
# speedup vs baseline: 1.2848x; 1.2848x over previous
"""GPT2 attention (B=2, S=2048, E=1024, H=16, interleaved QKV) on 8 trn2 NeuronCores.

Sharding: core c = 4*b + g handles batch b = c//4 and head group g = c%4
(heads 4g..4g+3): Megatron column-split of W_attn / row-split of W_proj,
data-parallel over batch. Host sums the 4 partial projection outputs per batch.

v2 design (throughput-oriented):
  - X^T is pre-transposed on the host; no PE transposes at all.
  - qk^T = W^T X^T (features on partitions); V computed directly in
    [token, dim] layout via x-stationary matmuls (no V transpose).
  - Scores S^T[sk,sq] per head with 64-deep contraction run 2-way
    concurrent on the two PE row-tiles (tile_position (0,0)/(64,0)),
    one head per half-array.
  - softmax exp is split between ACT (native Exp) and DVE (Schraudolph
    exponent-stuffing: int16(x*a+b) bitcast to f16) via a greedy
    load-balancing dispatcher. Causal masking of the 4 diagonal blocks
    is a 0/1 mask multiply on GPSIMD after the exp; diagonal score
    blocks only compute their valid (lower-trapezoid) width.
  - PV appends a ones column to V so PSUM row 64 = softmax denominators;
    1/den via an f16 bit-trick seed + one Newton step on the [1,512] row,
    broadcast with a rank-1 PE matmul kept in the same (64,128) tile mode
    as the score stream, then one DVE multiply.
  - projection PSUM is evacuated f16 alternating ACT/DVE, halving the
    output DMA.
"""
import numpy as np

import concourse.bass as bass
import concourse.bacc as bacc
import concourse.tile as tile
from concourse import mybir
from concourse.bass_utils import run_bass_kernel_spmd

F32 = mybir.dt.float32
F16 = mybir.dt.float16
I16 = mybir.dt.int16

B, S, E, H = 2, 2048, 1024, 16
HD = E // H            # 64
HPC = 4                # heads per core
CW = HPC * 3 * HD      # 768: W_attn cols per core
CP = HPC * HD          # 256: W_proj rows per core
NK = E // 128          # 8 contraction chunks over E
NSQ = S // 512         # 4 sq chunks of 512
NSK = S // 128         # 16 sk chunks of 128

# Schraudolph exp: exp(x) ~= bitcast_f16(round(x * SCH_A + SCH_B))
SCH_A = 1024.0 / float(np.log(2.0))
SCH_C = 44.0
SCH_B = 15.0 * 1024.0 - SCH_C
# f16 reciprocal seed: 1/d ~= bitcast_f16(RCP_K - bits_f16(d)), then one
# Newton step y1 = y0 * (2 - d*y0); max rel err ~3e-3
RCP_K = 30620.0

# dispatch cost model (ns) for the exp of a [128, w] block
ACT_COST = lambda w: (w + 352) / 1.2
DVE_COST = lambda w: 1.43 * w + 100.0  # fp32 input: half-rate DVE
DVE_NORM_EXTRA = 1200.0  # newton recip + bcast copy + mul per (head, J)

_cache = {}
_last_in_maps = None


def _build():
    from contextlib import ExitStack

    nc = bacc.Bacc("TRN2", target_bir_lowering=False, debug=False, num_devices=8)

    x_d = nc.dram_tensor("x", [NK, 128, S], F16, kind="ExternalInput").ap()
    wa_d = nc.dram_tensor("wa", [NK, 128, CW], F16, kind="ExternalInput").ap()
    ba_d = nc.dram_tensor("ba", [4, 128, 1], F32, kind="ExternalInput").ap()
    bv_d = nc.dram_tensor("bv", [1, 1, CP], F16, kind="ExternalInput").ap()
    wp_d = nc.dram_tensor("wp", [2, 128, E], F16, kind="ExternalInput").ap()
    out_d = nc.dram_tensor("out_t", [8, 128, S], F16, kind="ExternalOutput").ap()

    # canonical diagonal mask in reduced coords: keep (1) where g >= p
    gi = np.arange(512)[None, :]
    pi = np.arange(128)[:, None]
    maskg_np = (gi >= pi).astype(np.float16)
    maskg_d = nc.inline_tensor(maskg_np, name="maskg")
    ones1_d = nc.inline_tensor(np.ones((1, 128), np.float16), name="ones1")
    # rank-1 broadcast stationary: row 64 = ones, used in (64,128) tile mode
    oneblk_np = (np.arange(128)[:, None] == 64).astype(np.float16) * np.ones(
        (1, 128), np.float16
    )
    oneblk_d = nc.inline_tensor(oneblk_np, name="oneblk")

    Exp = mybir.ActivationFunctionType.Exp
    Ident = mybir.ActivationFunctionType.Identity
    Mult = mybir.AluOpType.mult
    Add = mybir.AluOpType.add

    with tile.TileContext(nc) as tc, ExitStack() as top:
        consts = top.enter_context(tc.tile_pool(name="consts", bufs=1))
        qk_pool = top.enter_context(tc.tile_pool(name="qkT", bufs=1))
        at_pool = top.enter_context(tc.tile_pool(name="attnT", bufs=1))
        wp_pool = top.enter_context(tc.tile_pool(name="wp", bufs=1))
        vb_pool = top.enter_context(tc.tile_pool(name="vb", bufs=1))
        xTp = top.enter_context(tc.tile_pool(name="xT", bufs=1))

        # ---- input DMAs -------------------------------------------------
        xT = [xTp.tile([128, S], F16, tag=f"xT{k}", name=f"xT{k}") for k in range(NK)]
        wa_t = consts.tile([128, NK, CW], F16)
        # wa chunk k is needed at the same time as xT[k]; put both early and
        # spread across queues.
        dq = [nc.sync, nc.scalar]
        for k in range(NK):
            nc.gpsimd.dma_start(out=wa_t[:, k, :], in_=wa_d[k])
            dq[k % 2].dma_start(out=xT[k][:], in_=x_d[k])
        wp_t = wp_pool.tile([128, 2, E], F16)
        for cc in range(2):
            nc.gpsimd.dma_start(out=wp_t[:, cc, :], in_=wp_d[cc])
        ba_t = consts.tile([128, 4], F32)
        for cc in range(4):
            nc.gpsimd.dma_start(out=ba_t[:, cc : cc + 1], in_=ba_d[cc])
        bv_t = consts.tile([1, CP], F16)
        nc.gpsimd.dma_start(out=bv_t[:], in_=bv_d[0])
        maskg_t = consts.tile([128, 512], F16)
        nc.gpsimd.dma_start(out=maskg_t[:], in_=maskg_d.ap())
        ones1_t = consts.tile([1, 128], F16)
        nc.gpsimd.dma_start(out=ones1_t[:], in_=ones1_d.ap())

        qkvT = [
            qk_pool.tile([128, S], F16, tag=f"qkT{cc}", name=f"qkT{cc}")
            for cc in range(4)
        ]
        attnT = [
            at_pool.tile([128, S], F16, tag=f"attnT{c}", name=f"attnT{c}")
            for c in range(2)
        ]
        # V with a ones column appended: [sk-chunk partitions, i, head, 65]
        vb4 = vb_pool.tile([128, NSK, HPC, 65], F16)
        nc.gpsimd.memset(vb4[:, :, :, 64:65], 1.0)

        # ---- phase 1: qk^T = W^T X^T, V = X Wv --------------------------
        with (
            tc.tile_pool(name="ps_mm", bufs=4, space="PSUM") as ps_mm,
            tc.tile_pool(name="ps_v", bufs=3, space="PSUM") as ps_v,
            tc.tile_pool(name="ps_b", bufs=1, space="PSUM") as ps_b,
        ):
            # v-bias broadcast (32-row tile mode; issued first so the rest
            # of phase 1 stays in 128x128 mode)
            psb = ps_b.tile([128, HPC, 64], F32, tag="pvb", name="pvb")
            nc.tensor.matmul(psb[:], ones1_t[0:1, :], bv_t[0:1, :], start=True, stop=True)
            bvbc = consts.tile([128, HPC, 64], F32)
            nc.vector.tensor_copy(bvbc[:], psb[:])

            for cc in range(4):
                pss = [
                    ps_mm.tile([128, 512], F32, tag="mm", name="mm_ps")
                    for _ in range(4)
                ]
                for k in range(NK):
                    lhs = wa_t[:, k, cc * 128 : (cc + 1) * 128]
                    for rc in range(4):
                        nc.tensor.matmul(
                            pss[rc][:],
                            lhs,
                            xT[k][:, rc * 512 : (rc + 1) * 512],
                            start=(k == 0),
                            stop=(k == NK - 1),
                        )
                for rc in range(4):
                    nc.scalar.activation(
                        qkvT[cc][:, rc * 512 : (rc + 1) * 512],
                        pss[rc][:],
                        Ident,
                        bias=ba_t[:, cc : cc + 1],
                        scale=0.125 if cc < 2 else 1.0,
                    )

            for i in range(NSK):
                psv = ps_v.tile([128, HPC, 64], F32, tag="pv", name="pv_ps")
                for k in range(NK):
                    nc.tensor.matmul(
                        psv[:],
                        xT[k][:, i * 128 : (i + 1) * 128],
                        wa_t[:, k, 512:768],
                        start=(k == 0),
                        stop=(k == NK - 1),
                    )
                nc.vector.tensor_add(vb4[:, i, :, 0:64], psv[:], bvbc[:])

        # ---- phase 2: per-head-pair attention ---------------------------
        with (
            tc.tile_pool(name="pp", bufs=36) as p_pool,
            tc.tile_pool(name="sm", bufs=4) as small,
            tc.tile_pool(name="rcf", bufs=1) as rcf_pool,
            tc.tile_pool(name="ps_s", bufs=4, space="PSUM") as ps_s,
            tc.tile_pool(name="ps_pv", bufs=1, space="PSUM") as ps_pv,
            tc.tile_pool(name="ps_bc", bufs=2, space="PSUM") as ps_bc,
        ):
            oneblk_t = consts.tile([128, 128], F16)
            nc.gpsimd.dma_start(out=oneblk_t[:], in_=oneblk_d.ap())
            # rcp rows ring: row 64 carries 1/den as f16 for the rank-1
            # broadcast matmul; rows 65..127 zeroed once (NaN guard: the
            # stationary is zero there and 0*NaN would poison the matmul)
            rcf_ring = []
            for n in range(4):
                t = rcf_pool.tile([128, 512], F16, tag=f"rcf{n}", name=f"rcf{n}")
                nc.vector.memset(t[64:128, :], 0.0)
                rcf_ring.append(t)

            act_t = 0.0
            dve_t = 0.0
            pending = []
            nidx = 0

            for pr in range(2):
                qT = qkvT[pr]
                kT = qkvT[2 + pr]
                for J in range(NSQ):
                    nblk = 4 * J + 4
                    sq = bass.ts(J, 512)
                    # block order: diagonal r=0..3 first, then off-diagonal
                    order = [4 * J + r for r in range(4)] + list(range(4 * J))
                    pblks = {}
                    nissued = 0
                    for i in order:
                        r = i - 4 * J
                        w = 512 if r < 0 else 512 - 128 * r
                        sqo = J * 512 + (0 if r < 0 else 128 * r)
                        for hh in range(2):
                            o = hh * 64
                            sps = ps_s.tile([128, 512], F32, tag="s", name="sps")
                            nc.tensor.matmul(
                                sps[:, 0:w],
                                kT[o : o + 64, i * 128 : (i + 1) * 128],
                                qT[o : o + 64, sqo : sqo + w],
                                start=True,
                                stop=True,
                                tile_position=(o, 0),
                            )
                            pt = p_pool.tile([128, 512], I16, tag="p", name="p")
                            pf = pt[:, 0:w].bitcast(F16)
                            ca, cd = ACT_COST(w), DVE_COST(w)
                            if act_t + ca <= dve_t + cd:
                                act_t += ca
                                nc.scalar.activation(pf, sps[:, 0:w], Exp)
                            else:
                                dve_t += cd
                                nc.vector.tensor_scalar(
                                    pt[:, 0:w], sps[:, 0:w], SCH_A, SCH_B, Mult, Add
                                )
                            if r >= 0:
                                nc.gpsimd.tensor_mul(pf, pf, maskg_t[:, 0:w])
                            pblks[(hh, i)] = pt
                        nissued += 1
                        # the previous group's broadcast matmuls + final muls
                        # land here: same (64,128) PE tile mode as the S
                        # stream, and their DVE inputs are ready by now
                        if nissued == 2 and pending:
                            for fn in pending:
                                fn()
                            pending = []
                    if pending:
                        for fn in pending:
                            fn()
                        pending = []

                    for hh in range(2):
                        o = hh * 64
                        pv = ps_pv.tile(
                            [65, 512], F32, tag=f"pv{hh}", name=f"pv{hh}"
                        )
                        for n, i in enumerate(order):
                            r = i - 4 * J
                            w = 512 if r < 0 else 512 - 128 * r
                            co = 0 if r < 0 else 128 * r
                            nc.tensor.matmul(
                                pv[:, co : co + w],
                                vb4[:, i, 2 * pr + hh, :],
                                pblks[(hh, i)][:, 0:w].bitcast(F16),
                                start=(n == 0),
                                stop=(n == nblk - 1),
                            )

                        # 1/den via f16 bit-trick seed + one Newton step, all
                        # on the [1,512] denominator row (partition 64)
                        rcf = rcf_ring[nidx % 4]
                        nidx += 1
                        nc.vector.tensor_copy(rcf[64:65, :], pv[64:65, :])
                        y0 = small.tile([128, 512], I16, tag="y0", name="y0")
                        nc.vector.tensor_scalar(
                            y0[64:65, :], rcf[64:65, :].bitcast(I16),
                            -1.0, RCP_K, Mult, Add,
                        )
                        t1 = small.tile([128, 512], F16, tag="t1", name="t1")
                        nc.vector.tensor_mul(
                            t1[64:65, :], rcf[64:65, :], y0[64:65, :].bitcast(F16)
                        )
                        nc.vector.tensor_scalar(
                            t1[64:65, :], t1[64:65, :], -1.0, 2.0, Mult, Add
                        )
                        nc.vector.tensor_mul(
                            rcf[64:65, :], y0[64:65, :].bitcast(F16), t1[64:65, :]
                        )
                        dve_t += DVE_NORM_EXTRA

                        def norm(pv=pv, o=o, sq=sq, pr=pr, rcf=rcf):
                            bcp = ps_bc.tile([128, 512], F32, tag="bc", name="bcp")
                            nc.tensor.matmul(
                                bcp[:],
                                oneblk_t[64:128, :],
                                rcf[64:128, :],
                                start=True,
                                stop=True,
                                tile_position=(64, 0),
                            )
                            bcs = small.tile([64, 512], F32, tag="bcs", name="bcs")
                            nc.vector.tensor_copy(bcs[:], bcp[0:64, :])
                            nc.vector.tensor_mul(
                                attnT[pr][o : o + 64, sq], pv[0:64, :], bcs[:]
                            )

                        pending.append(norm)
            for fn in pending:
                fn()

        # ---- phase 3: projection ----------------------------------------
        with (
            tc.tile_pool(name="ob", bufs=6) as ob_pool,
            tc.tile_pool(name="ps_mm2", bufs=4, space="PSUM") as ps_mm2,
        ):
            for eo in range(8):
                pss = [
                    ps_mm2.tile([128, 512], F32, tag="mm2", name="mm2_ps")
                    for _ in range(4)
                ]
                for cc in range(2):
                    lhs = wp_t[:, cc, eo * 128 : (eo + 1) * 128]
                    for Jq in range(4):
                        nc.tensor.matmul(
                            pss[Jq][:],
                            lhs,
                            attnT[cc][:, Jq * 512 : (Jq + 1) * 512],
                            start=(cc == 0),
                            stop=(cc == 1),
                        )
                for Jq in range(4):
                    ob = ob_pool.tile([128, 512], F16, tag="ob", name="ob")
                    if Jq % 2 == 0:
                        nc.scalar.copy(ob[:], pss[Jq][:])
                    else:
                        nc.vector.tensor_copy(ob[:], pss[Jq][:])
                    eng = nc.sync if (eo * 4 + Jq) % 2 == 0 else nc.scalar
                    eng.dma_start(
                        out=out_d[eo][:, Jq * 512 : (Jq + 1) * 512], in_=ob[:]
                    )

    nc.compile()
    return nc


def _col_perm(g):
    """Per-core W_attn column permutation: [q0..q3 | k0..k3 | v0..v3]."""
    cols = []
    for t in range(3):          # q, k, v
        for h in range(HPC):
            base = (4 * g + h) * 3 * HD + t * HD
            cols.append(np.arange(base, base + HD))
    return np.concatenate(cols)


def kernel(hidden_states, W_attn, b_attn, W_proj, b_proj):
    hidden_states = np.asarray(hidden_states, np.float32)
    W_attn = np.asarray(W_attn, np.float32)
    b_attn = np.asarray(b_attn, np.float32)
    W_proj = np.asarray(W_proj, np.float32)
    b_proj = np.asarray(b_proj, np.float32)

    if "nc" not in _cache:
        _cache["nc"] = _build()
    nc = _cache["nc"]

    # q columns (first 256 of the permuted layout) have scale 1/8 folded into
    # the PSUM->SBUF copy; bias is added after the scale, so pre-scale it.
    bias_scale = np.ones(2 * CP, np.float32)
    bias_scale[:CP] = 0.125

    in_maps = []
    for c in range(8):
        b, g = divmod(c, 4)
        perm = _col_perm(g)
        wa = np.ascontiguousarray(W_attn[:, perm])
        ba = (b_attn[perm][: 2 * CP] * bias_scale).astype(np.float32)
        bv = b_attn[perm][2 * CP :].astype(np.float16)
        wp = np.ascontiguousarray(W_proj[g * CP : (g + 1) * CP, :])
        xT = np.ascontiguousarray(hidden_states[b].T).astype(np.float16)
        in_maps.append(
            {
                "x": xT.reshape(NK, 128, S),
                "wa": wa.astype(np.float16).reshape(NK, 128, CW),
                "ba": ba.reshape(4, 128, 1),
                "bv": bv.reshape(1, 1, CP),
                "wp": wp.astype(np.float16).reshape(2, 128, E),
            }
        )

    global _last_in_maps
    _last_in_maps = in_maps
    res = run_bass_kernel_spmd(nc, in_maps, list(range(8)))

    out = np.zeros((B, S, E), np.float32)
    for c in range(8):
        b = c // 4
        out[b] += res.results[c]["out_t"].reshape(E, S).astype(np.float32).T
    out += b_proj
    return out


# revision 23
# speedup vs baseline: 1.4827x; 1.1540x over previous
"""GPT2 attention (B=2, S=2048, E=1024, H=16, interleaved QKV) on 8 trn2 NeuronCores.

Sharding: core c = 4*b + g handles batch b = c//4 and head group g = c%4
(heads 4g..4g+3): Megatron column-split of W_attn / row-split of W_proj,
data-parallel over batch. Host sums the 4 partial projection outputs per batch.

v2 design (throughput-oriented):
  - X^T is pre-transposed on the host; no PE transposes at all.
  - qk^T = W^T X^T (features on partitions); V computed directly in
    [token, dim] layout via x-stationary matmuls (no V transpose).
  - Scores S^T[sk,sq] per head with 64-deep contraction run 2-way
    concurrent on the two PE row-tiles (tile_position (0,0)/(64,0)),
    one head per half-array.
  - softmax exp is split between ACT (native Exp) and DVE (Schraudolph
    exponent-stuffing: int16(x*a+b) bitcast to f16) via a greedy
    load-balancing dispatcher. Causal masking of the 4 diagonal blocks
    is a 0/1 mask multiply on GPSIMD after the exp; diagonal score
    blocks only compute their valid (lower-trapezoid) width.
  - PV appends a ones column to V so PSUM row 64 = softmax denominators;
    1/den via an f16 bit-trick seed + one Newton step on the [1,512] row,
    broadcast with a rank-1 PE matmul kept in the same (64,128) tile mode
    as the score stream, then one DVE multiply.
  - projection PSUM is evacuated f16 alternating ACT/DVE, halving the
    output DMA.
"""
import numpy as np

import concourse.bass as bass
import concourse.bacc as bacc
import concourse.tile as tile
from concourse import mybir
from concourse.bass_utils import run_bass_kernel_spmd

F32 = mybir.dt.float32
F16 = mybir.dt.float16
I16 = mybir.dt.int16
U16 = mybir.dt.uint16

B, S, E, H = 2, 2048, 1024, 16
HD = E // H            # 64
HPC = 4                # heads per core
CW = HPC * 3 * HD      # 768: W_attn cols per core
CP = HPC * HD          # 256: W_proj rows per core
NK = E // 128          # 8 contraction chunks over E
NSQ = S // 512         # 4 sq chunks of 512
NSK = S // 128         # 16 sk chunks of 128

# Schraudolph exp: exp(x) ~= bitcast_f16(round(x * SCH_A + SCH_B)); the u16
# output convert saturates negatives to 0, so masked entries (biased by
# -60000 via the fused mask operand) become exactly +0.0
SCH_A = 1024.0 / float(np.log(2.0))
SCH_B = 15312.0  # 15360 - 48, f16-exact so the mask constant tiles match
MASKED = -60000.0
# f16 reciprocal seed: 1/d ~= bitcast_f16(RCP_K - bits_f16(d)), then one
# Newton step y1 = y0 * (2 - d*y0); max rel err ~3e-3
RCP_K = 30620.0

# dispatch cost model (ns) for the exp of a [128, n]-column pair tile
ACT_COST = lambda n: (n + 352) / 1.2
DVE_COST = lambda n: 0.52 * n + 300.0
DVE_NORM_EXTRA = 2600.0  # den copies + newton + bcs + muls per (pair, J)

_cache = {}
_last_in_maps = None


def _build():
    from contextlib import ExitStack

    nc = bacc.Bacc("TRN2", target_bir_lowering=False, debug=False, num_devices=8)

    x_d = nc.dram_tensor("x", [NK, 128, S], F16, kind="ExternalInput").ap()
    wa_d = nc.dram_tensor("wa", [NK, 128, CW], F16, kind="ExternalInput").ap()
    ba_d = nc.dram_tensor("ba", [4, 128, 1], F32, kind="ExternalInput").ap()
    bv_d = nc.dram_tensor("bv", [1, 1, CP], F16, kind="ExternalInput").ap()
    wp_d = nc.dram_tensor("wp", [2, 128, E], F16, kind="ExternalInput").ap()
    out_d = nc.dram_tensor("out_t", [8, 128, S], F16, kind="ExternalOutput").ap()

    # fused-mask bias tiles for the diagonal score blocks, pair layout:
    # [h0: 0..w | gap | h1: 512..512+w | gap]; reduced coords g (block col).
    # keep (bias SCH_B) where g >= p, else MASKED (-> exp saturates to 0).
    gi = np.arange(512)[None, :]
    pi = np.arange(128)[:, None]
    maskB_d = []
    for r in range(4):
        w = 512 - 128 * r
        half = np.full((128, 512), MASKED, np.float16)
        keep = (gi < w) & (gi >= pi)
        half[keep] = SCH_B
        maskB_d.append(
            nc.inline_tensor(
                np.concatenate([half, half], axis=1), name=f"maskB{r}"
            )
        )
    ones1_d = nc.inline_tensor(np.ones((1, 128), np.float16), name="ones1")
    # rank-1 broadcast stationaries: row 64 (and 65) = ones, used in the
    # (64,128) tile mode so they do not break the score-stream mode
    oneblk_np = np.zeros((128, 128), np.float16)
    oneblk_np[64, :] = 1.0
    oneblk96_np = np.zeros((128, 128), np.float16)
    oneblk96_np[96, :] = 1.0
    oneblk_d = nc.inline_tensor(oneblk_np, name="oneblk")
    oneblk96_d = nc.inline_tensor(oneblk96_np, name="oneblk96")

    Exp = mybir.ActivationFunctionType.Exp
    Ident = mybir.ActivationFunctionType.Identity
    Mult = mybir.AluOpType.mult
    Add = mybir.AluOpType.add

    with tile.TileContext(nc) as tc, ExitStack() as top:
        consts = top.enter_context(tc.tile_pool(name="consts", bufs=1))
        qk_pool = top.enter_context(tc.tile_pool(name="qkT", bufs=1))
        at_pool = top.enter_context(tc.tile_pool(name="attnT", bufs=1))
        wp_pool = top.enter_context(tc.tile_pool(name="wp", bufs=1))
        vb_pool = top.enter_context(tc.tile_pool(name="vb", bufs=1))
        xTp = top.enter_context(tc.tile_pool(name="xT", bufs=1))

        # ---- input DMAs -------------------------------------------------
        xT = [xTp.tile([128, S], F16, tag=f"xT{k}", name=f"xT{k}") for k in range(NK)]
        wa_t = consts.tile([128, NK, CW], F16)
        # wa chunk k is needed at the same time as xT[k]; put both early and
        # spread across queues.
        dq = [nc.sync, nc.scalar]
        for k in range(NK):
            nc.gpsimd.dma_start(out=wa_t[:, k, :], in_=wa_d[k])
            dq[k % 2].dma_start(out=xT[k][:], in_=x_d[k])
        wp_t = wp_pool.tile([128, 2, E], F16)
        for cc in range(2):
            nc.gpsimd.dma_start(out=wp_t[:, cc, :], in_=wp_d[cc])
        ba_t = consts.tile([128, 4], F32)
        for cc in range(4):
            nc.gpsimd.dma_start(out=ba_t[:, cc : cc + 1], in_=ba_d[cc])
        bv_t = consts.tile([1, CP], F16)
        nc.gpsimd.dma_start(out=bv_t[:], in_=bv_d[0])
        maskB_t = consts.tile([128, 4, 1024], F16)
        for r in range(4):
            nc.gpsimd.dma_start(out=maskB_t[:, r, :], in_=maskB_d[r].ap())
        ones1_t = consts.tile([1, 128], F16)
        nc.gpsimd.dma_start(out=ones1_t[:], in_=ones1_d.ap())

        qkvT = [
            qk_pool.tile([128, S], F16, tag=f"qkT{cc}", name=f"qkT{cc}")
            for cc in range(4)
        ]
        attnT = [
            at_pool.tile([128, S], F16, tag=f"attnT{c}", name=f"attnT{c}")
            for c in range(2)
        ]
        # V with a ones column appended: [sk-chunk partitions, i, head, 65]
        vb4 = vb_pool.tile([128, NSK, HPC, 65], F16)
        nc.gpsimd.memset(vb4[:, :, :, 64:65], 1.0)

        # ---- phase 1: qk^T = W^T X^T, V = X Wv --------------------------
        with (
            tc.tile_pool(name="ps_mm", bufs=4, space="PSUM") as ps_mm,
            tc.tile_pool(name="ps_v", bufs=3, space="PSUM") as ps_v,
            tc.tile_pool(name="ps_b", bufs=1, space="PSUM") as ps_b,
        ):
            # v-bias broadcast (32-row tile mode; issued first so the rest
            # of phase 1 stays in 128x128 mode)
            psb = ps_b.tile([128, HPC, 64], F32, tag="pvb", name="pvb")
            nc.tensor.matmul(psb[:], ones1_t[0:1, :], bv_t[0:1, :], start=True, stop=True)
            bvbc = consts.tile([128, HPC, 64], F32)
            nc.vector.tensor_copy(bvbc[:], psb[:])

            for cc in range(4):
                pss = [
                    ps_mm.tile([128, 512], F32, tag="mm", name="mm_ps")
                    for _ in range(4)
                ]
                for k in range(NK):
                    lhs = wa_t[:, k, cc * 128 : (cc + 1) * 128]
                    for rc in range(4):
                        nc.tensor.matmul(
                            pss[rc][:],
                            lhs,
                            xT[k][:, rc * 512 : (rc + 1) * 512],
                            start=(k == 0),
                            stop=(k == NK - 1),
                        )
                for rc in range(4):
                    nc.scalar.activation(
                        qkvT[cc][:, rc * 512 : (rc + 1) * 512],
                        pss[rc][:],
                        Ident,
                        bias=ba_t[:, cc : cc + 1],
                        scale=0.125 if cc < 2 else 1.0,
                    )

            for i in range(NSK):
                psv = ps_v.tile([128, HPC, 64], F32, tag="pv", name="pv_ps")
                for k in range(NK):
                    nc.tensor.matmul(
                        psv[:],
                        xT[k][:, i * 128 : (i + 1) * 128],
                        wa_t[:, k, 512:768],
                        start=(k == 0),
                        stop=(k == NK - 1),
                    )
                nc.vector.tensor_add(vb4[:, i, :, 0:64], psv[:], bvbc[:])

        # ---- phase 2: per-head-pair attention ---------------------------
        # Both heads of a pair share one [128,1024] score tile (one bank per
        # head), so every exp call covers two blocks. Diagonal blocks use the
        # DVE fused op (x*A + maskB) with u16 saturation; off-diagonal blocks
        # are load-balanced between ACT Exp and DVE Schraudolph.
        with (
            tc.tile_pool(name="pp", bufs=18) as p_pool,
            tc.tile_pool(name="sm", bufs=4) as small,
            tc.tile_pool(name="rcf", bufs=1) as rcf_pool,
            tc.tile_pool(name="ps_s", bufs=2, space="PSUM") as ps_s,
            tc.tile_pool(name="ps_pv", bufs=1, space="PSUM") as ps_pv,
            tc.tile_pool(name="ps_bc", bufs=2, space="PSUM") as ps_bc,
        ):
            oneblk_t = consts.tile([128, 128], F16)
            nc.gpsimd.dma_start(out=oneblk_t[:], in_=oneblk_d.ap())
            oneblk96_t = consts.tile([128, 128], F16)
            nc.gpsimd.dma_start(out=oneblk96_t[:], in_=oneblk96_d.ap())
            # rcp rows ring: rows 64/96 carry 1/den (h0/h1) as f16 for the
            # rank-1 broadcast matmuls (engine partition starts must be
            # 32-aligned); other rows zeroed once (NaN guard)
            # guard rows start at 1.0: the newton ops iterate y <- ~1/y on
            # them, for which 1.0 is a stable bounded fixed point (0.0 would
            # blow up to inf and 0*inf would NaN the broadcast matmul)
            rcf_ring = []
            for n in range(4):
                t = rcf_pool.tile([128, 512], F16, tag=f"rcf{n}", name=f"rcf{n}")
                nc.vector.memset(t[64:128, :], 1.0)
                rcf_ring.append(t)

            act_t = 0.0
            dve_t = 0.0
            pending = []
            nidx = 0

            for pr in range(2):
                qT = qkvT[pr]
                kT = qkvT[2 + pr]
                for J in range(NSQ):
                    nblk = 4 * J + 4
                    sq = bass.ts(J, 512)
                    # block order: diagonal r=0..3 first, then off-diagonal
                    order = [4 * J + r for r in range(4)] + list(range(4 * J))
                    pblks = {}
                    nissued = 0
                    for i in order:
                        r = i - 4 * J
                        w = 512 if r < 0 else 512 - 128 * r
                        sqo = J * 512 + (0 if r < 0 else 128 * r)
                        stile = ps_s.tile([128, 1024], F32, tag="s", name="sps")
                        for hh in range(2):
                            o = hh * 64
                            nc.tensor.matmul(
                                stile[:, hh * 512 : hh * 512 + w],
                                kT[o : o + 64, i * 128 : (i + 1) * 128],
                                qT[o : o + 64, sqo : sqo + w],
                                start=True,
                                stop=True,
                                tile_position=(o, 0),
                            )
                        pt = p_pool.tile([128, 1024], U16, tag="p", name="p")
                        if r >= 0:
                            dve_t += DVE_COST(1024)
                            nc.vector.scalar_tensor_tensor(
                                out=pt[:],
                                in0=stile[:],
                                scalar=SCH_A,
                                in1=maskB_t[:, r, :],
                                op0=Mult,
                                op1=Add,
                            )
                        else:
                            ca, cd = ACT_COST(1024), DVE_COST(1024)
                            if act_t + ca <= dve_t + cd:
                                act_t += ca
                                nc.scalar.activation(
                                    pt[:].bitcast(F16), stile[:], Exp
                                )
                            else:
                                dve_t += cd
                                nc.vector.tensor_scalar(
                                    pt[:], stile[:], SCH_A, SCH_B, Mult, Add
                                )
                        pblks[i] = pt
                        nissued += 1
                        # the previous group's broadcast matmuls + final muls
                        # land here: same (64,128) PE tile mode as the S
                        # stream, and their DVE inputs are ready by now
                        if nissued == 2 and pending:
                            for fn in pending:
                                fn()
                            pending = []
                    if pending:
                        for fn in pending:
                            fn()
                        pending = []

                    pvp = ps_pv.tile([65, 1024], F32, tag="pv", name="pvp")
                    for hh in range(2):
                        for n, i in enumerate(order):
                            r = i - 4 * J
                            w = 512 if r < 0 else 512 - 128 * r
                            co = hh * 512 + (0 if r < 0 else 128 * r)
                            nc.tensor.matmul(
                                pvp[:, co : co + w],
                                vb4[:, i, 2 * pr + hh, :],
                                pblks[i][:, hh * 512 : hh * 512 + w].bitcast(F16),
                                start=(n == 0),
                                stop=(n == nblk - 1),
                            )

                    # 1/den for both heads: f16 bit-trick seed + one Newton
                    # step on rcf rows 64 (h0) and 96 (h1); the ops run over
                    # the whole [64:128] row block (32-aligned starts) — the
                    # other rows turn into finite garbage that the zero rows
                    # of the broadcast stationary annihilate
                    rcf = rcf_ring[nidx % 4]
                    nidx += 1
                    nc.vector.tensor_copy(rcf[64:65, :], pvp[64:65, 0:512])
                    nc.vector.tensor_copy(rcf[96:97, :], pvp[64:65, 512:1024])
                    y0 = small.tile([128, 512], I16, tag="y0", name="y0")
                    nc.vector.tensor_scalar(
                        y0[64:128, :], rcf[64:128, :].bitcast(I16),
                        -1.0, RCP_K, Mult, Add,
                    )
                    t1 = small.tile([128, 512], F16, tag="t1", name="t1")
                    nc.vector.tensor_mul(
                        t1[64:128, :], rcf[64:128, :], y0[64:128, :].bitcast(F16)
                    )
                    nc.vector.tensor_scalar(
                        t1[64:128, :], t1[64:128, :], -1.0, 2.0, Mult, Add
                    )
                    nc.vector.tensor_mul(
                        rcf[64:128, :], y0[64:128, :].bitcast(F16), t1[64:128, :]
                    )
                    dve_t += DVE_NORM_EXTRA

                    def norm(pvp=pvp, sq=sq, pr=pr, rcf=rcf):
                        for hh, blk in ((0, oneblk_t), (1, oneblk96_t)):
                            o = hh * 64
                            bcp = ps_bc.tile(
                                [128, 512], F32, tag="bc", name="bcp"
                            )
                            nc.tensor.matmul(
                                bcp[:],
                                blk[64:128, :],
                                rcf[64:128, :],
                                start=True,
                                stop=True,
                                tile_position=(64, 0),
                            )
                            bcs = small.tile(
                                [64, 512], F32, tag="bcs", name="bcs"
                            )
                            nc.vector.tensor_copy(bcs[:], bcp[0:64, :])
                            nc.vector.tensor_mul(
                                attnT[pr][o : o + 64, sq],
                                pvp[0:64, hh * 512 : (hh + 1) * 512],
                                bcs[:],
                            )

                    pending.append(norm)
            for fn in pending:
                fn()

        # ---- phase 3: projection ----------------------------------------
        with (
            tc.tile_pool(name="ob", bufs=6) as ob_pool,
            tc.tile_pool(name="ps_mm2", bufs=4, space="PSUM") as ps_mm2,
        ):
            for eo in range(8):
                pss = [
                    ps_mm2.tile([128, 512], F32, tag="mm2", name="mm2_ps")
                    for _ in range(4)
                ]
                for cc in range(2):
                    lhs = wp_t[:, cc, eo * 128 : (eo + 1) * 128]
                    for Jq in range(4):
                        nc.tensor.matmul(
                            pss[Jq][:],
                            lhs,
                            attnT[cc][:, Jq * 512 : (Jq + 1) * 512],
                            start=(cc == 0),
                            stop=(cc == 1),
                        )
                for Jq in range(4):
                    ob = ob_pool.tile([128, 512], F16, tag="ob", name="ob")
                    if Jq % 2 == 0:
                        nc.scalar.copy(ob[:], pss[Jq][:])
                    else:
                        nc.vector.tensor_copy(ob[:], pss[Jq][:])
                    eng = nc.sync if (eo * 4 + Jq) % 2 == 0 else nc.scalar
                    eng.dma_start(
                        out=out_d[eo][:, Jq * 512 : (Jq + 1) * 512], in_=ob[:]
                    )

    nc.compile()
    return nc


def _col_perm(g):
    """Per-core W_attn column permutation: [q0..q3 | k0..k3 | v0..v3]."""
    cols = []
    for t in range(3):          # q, k, v
        for h in range(HPC):
            base = (4 * g + h) * 3 * HD + t * HD
            cols.append(np.arange(base, base + HD))
    return np.concatenate(cols)


def kernel(hidden_states, W_attn, b_attn, W_proj, b_proj):
    hidden_states = np.asarray(hidden_states, np.float32)
    W_attn = np.asarray(W_attn, np.float32)
    b_attn = np.asarray(b_attn, np.float32)
    W_proj = np.asarray(W_proj, np.float32)
    b_proj = np.asarray(b_proj, np.float32)

    if "nc" not in _cache:
        _cache["nc"] = _build()
    nc = _cache["nc"]

    # q columns (first 256 of the permuted layout) have scale 1/8 folded into
    # the PSUM->SBUF copy; bias is added after the scale, so pre-scale it.
    bias_scale = np.ones(2 * CP, np.float32)
    bias_scale[:CP] = 0.125

    in_maps = []
    for c in range(8):
        b, g = divmod(c, 4)
        perm = _col_perm(g)
        wa = np.ascontiguousarray(W_attn[:, perm])
        ba = (b_attn[perm][: 2 * CP] * bias_scale).astype(np.float32)
        bv = b_attn[perm][2 * CP :].astype(np.float16)
        wp = np.ascontiguousarray(W_proj[g * CP : (g + 1) * CP, :])
        xT = np.ascontiguousarray(hidden_states[b].T).astype(np.float16)
        in_maps.append(
            {
                "x": xT.reshape(NK, 128, S),
                "wa": wa.astype(np.float16).reshape(NK, 128, CW),
                "ba": ba.reshape(4, 128, 1),
                "bv": bv.reshape(1, 1, CP),
                "wp": wp.astype(np.float16).reshape(2, 128, E),
            }
        )

    global _last_in_maps
    _last_in_maps = in_maps
    res = run_bass_kernel_spmd(nc, in_maps, list(range(8)))

    out = np.zeros((B, S, E), np.float32)
    for c in range(8):
        b = c // 4
        out[b] += res.results[c]["out_t"].reshape(E, S).astype(np.float32).T
    out += b_proj
    return out
